# revision 1
# baseline (speedup 1.0000x reference)
"""Trainium2 Bass kernel for nn_Equalize (soft histogram equalization).

Algorithm (per core; 8 cores, each owns a quarter of one of the 2 images):
  1. Fine histogram (8160 bins) of the core's 65536 pixels via two-level
     one-hot (96 x 85) outer-product matmuls accumulated in PSUM.
  2. AllReduce the fine histogram across the 4 cores of each image.
  3. Coarse 256-bin soft histogram = Toeplitz window-conv of the fine
     histogram with the Gaussian kernel (DVE mul+reduce on strided DMA views).
  4. cdf via triangular matmul; normalize to cdfn.
  5. G lookup table (4096 entries): G(v) = sum_j k(v-b_j)*cdfn[j] / sum_j k(v-b_j)
     computed with a small Toeplitz matmul (window of 32 bins).
  6. Per-pixel output = G[round(x*4080)] via GPSIMD ap_gather.

The output of the reference only depends on a pixel through the smooth 1-D
function G, so a fine table lookup reproduces it to ~1e-4.
"""
import os
import numpy as np

import concourse.bass as bass
import concourse.mybir as mybir
import concourse.tile as tile
import concourse.bacc as bacc
from concourse.bass_utils import run_bass_kernel_spmd

F32 = mybir.dt.float32
I32 = mybir.dt.int32
I16 = mybir.dt.int16
BF16 = mybir.dt.bfloat16

B, H, W = 2, 512, 512
N_CORES = 8
QUARTER = H // 4 * W            # 65536 pixels per core
N_BINS = 256
TAU = 0.01
C = 1.0 / (2.0 * TAU * TAU)     # 5000
SQC = float(np.sqrt(C))
NF = 8160                        # fine-hist resolution (32*255)
NHI, NLO = 96, 85                # NF = NHI*NLO
TWIN = 1024                      # conv window (fine bins)
PADL = TWIN // 2                 # 512
HF_LEN = PADL + NF + (TWIN // 2 + 32)   # 9216, padded fine hist
TGRID = 4080                     # G-table grid (16*255)
MSUB = 16                        # table sub-samples per bin
TLEN = 4096                      # table allocation (num_elems)
KWIN = 32                        # G window in coarse bins
NTILE = QUARTER // 128           # 512 pixel tiles
NPX_GRP = QUARTER // 8           # 8192 pixels per gpsimd core group


def mk_ap(handle_ap, offset, pairs):
    import dataclasses
    return dataclasses.replace(handle_ap, offset=offset, ap=list(pairs))


def build_nc(stage=3):
    stage = int(os.environ.get("KERNEL_STAGE", stage))
    nc = bacc.Bacc()
    x_dram = nc.declare_dram_parameter("x", [QUARTER], F32, isOutput=False)
    out_dram = nc.declare_dram_parameter("out", [QUARTER], F32, isOutput=True)

    hf_dram = nc.dram_tensor("hf_local", [HF_LEN], F32)
    hf_red = nc.dram_tensor("hf_red", [HF_LEN], F32)
    cpad_dram = nc.dram_tensor("cpad", [N_BINS + KWIN], F32)    # 288
    vpad_dram = nc.dram_tensor("vpad", [N_BINS + KWIN], F32)
    gtab_dram = nc.dram_tensor("gtab", [TLEN], F32)

    with tile.TileContext(nc) as tc:
        with (
            tc.tile_pool(name="big", bufs=1) as big,
            tc.tile_pool(name="oh", bufs=4) as ohp,
            tc.tile_pool(name="small", bufs=1) as sm,
            tc.tile_pool(name="psum", bufs=1, space="PSUM") as psp,
        ):
            # ---------------- constants ----------------
            iota_hi_i = sm.tile([128, NHI], I32)
            nc.gpsimd.iota(iota_hi_i[:], pattern=[[1, NHI]], base=0, channel_multiplier=0)
            iota_hi = sm.tile([128, NHI], BF16)
            nc.vector.tensor_copy(iota_hi[:], iota_hi_i[:])

            iota_lo_i = sm.tile([128, NLO], I32)
            nc.gpsimd.iota(iota_lo_i[:], pattern=[[1, NLO]], base=0, channel_multiplier=0)
            iota_lo = sm.tile([128, NLO], BF16)
            nc.vector.tensor_copy(iota_lo[:], iota_lo_i[:])

            # kw[t] = exp(-C*((t-511.5)/NF)^2), replicated per partition
            kw_i = sm.tile([128, TWIN], I32)
            nc.gpsimd.iota(kw_i[:], pattern=[[1, TWIN]], base=0, channel_multiplier=0)
            kw_f = sm.tile([128, TWIN], F32)
            nc.vector.tensor_copy(kw_f[:], kw_i[:])
            kw_sq = sm.tile([128, TWIN], F32)
            bias_kw = sm.tile([128, 1], F32)
            nc.vector.memset(bias_kw[:], -SQC * (TWIN / 2 - 0.5) / NF)
            nc.scalar.activation(kw_sq[:], kw_f[:], mybir.ActivationFunctionType.Square,
                                 bias=bias_kw[:], scale=SQC / NF)
            kw = sm.tile([128, TWIN], F32)
            nc.scalar.activation(kw[:], kw_sq[:], mybir.ActivationFunctionType.Exp,
                                 scale=-1.0)

            # W_win lhsT [32 k, 16 m]: exp(-C*((m + 256 - 16k)/TGRID)^2)
            ww_i = sm.tile([KWIN, MSUB], I32)
            nc.gpsimd.iota(ww_i[:], pattern=[[1, MSUB]], base=256, channel_multiplier=-16)
            ww_f = sm.tile([KWIN, MSUB], F32)
            nc.vector.tensor_copy(ww_f[:], ww_i[:])
            ww_sq = sm.tile([KWIN, MSUB], F32)
            bias_z32 = sm.tile([KWIN, 1], F32)
            nc.vector.memset(bias_z32[:], 0.0)
            nc.scalar.activation(ww_sq[:], ww_f[:], mybir.ActivationFunctionType.Square,
                                 bias=bias_z32[:], scale=SQC / TGRID)
            ww = sm.tile([KWIN, MSUB], F32)
            nc.scalar.activation(ww[:], ww_sq[:], mybir.ActivationFunctionType.Exp,
                                 scale=-1.0)

            # triangular matrices for cumsum: iota j-k
            tri_i = sm.tile([128, N_BINS], I16)
            nc.gpsimd.iota(tri_i[:], pattern=[[1, N_BINS]], base=0, channel_multiplier=-1)
            tri0 = sm.tile([128, N_BINS], F32)
            nc.vector.tensor_scalar(tri0[:], tri_i[:], 0.0, None, mybir.AluOpType.is_ge)
            tri1 = sm.tile([128, N_BINS], F32)
            nc.vector.tensor_scalar(tri1[:], tri_i[:], 128.0, None, mybir.AluOpType.is_ge)

            # zero row for DRAM padding; ones/valid row
            z_row = sm.tile([1, PADL + TWIN // 2 + 32], F32)
            nc.vector.memset(z_row[:], 0.0)
            vp_row = sm.tile([1, N_BINS + KWIN], F32)
            nc.vector.memset(vp_row[:], 0.0)
            nc.vector.memset(vp_row[:, KWIN // 2:KWIN // 2 + N_BINS], 1.0)
            nc.sync.dma_start(vpad_dram.ap(), vp_row[:])

            # ------------- early library warm-up for ap_gather -------------
            warm_tab = sm.tile([128, 4], F32)
            nc.vector.memset(warm_tab[:], 0.0)
            warm_idx = sm.tile([128, 1], I16)
            nc.vector.memset(warm_idx[:], 0)
            warm_out = sm.tile([128, 16], F32)
            nc.gpsimd.ap_gather(
                warm_out[:].rearrange("c (n d) -> c n d", d=1),
                warm_tab[:].rearrange("c (n d) -> c n d", d=1),
                warm_idx[:], channels=128, num_elems=4, d=1, num_idxs=16)

            # ---------------- phase 1: fine histogram ----------------
            x_sb = big.tile([128, NTILE], F32)
            nc.sync.dma_start(x_sb[:], x_dram.ap().rearrange("(p t) -> p t", p=128))

            # HW f32->int converts round-to-nearest-even, so floor(v) is
            # round(v - 0.5) (ties land on even, a half-fine-bin jitter).
            v_sb = big.tile([128, NTILE], F32)
            nc.vector.tensor_scalar(v_sb[:], x_sb[:], float(NHI), None, mybir.AluOpType.mult)
            hi_i = big.tile([128, NTILE], I32)
            nc.vector.tensor_scalar(hi_i[:], v_sb[:], 0.5, None, mybir.AluOpType.subtract)
            hi_f = big.tile([128, NTILE], F32)
            nc.vector.tensor_copy(hi_f[:], hi_i[:])
            fr_sb = big.tile([128, NTILE], F32)
            nc.vector.tensor_tensor(fr_sb[:], v_sb[:], hi_f[:], mybir.AluOpType.subtract)
            lo_i = big.tile([128, NTILE], I32)
            nc.vector.tensor_scalar(lo_i[:], fr_sb[:], float(NLO), 0.5,
                                    mybir.AluOpType.mult, mybir.AluOpType.subtract)
            lo_f = big.tile([128, NTILE], F32)
            nc.vector.tensor_copy(lo_f[:], lo_i[:])

            hf_psum = psp.tile([NHI, NLO], F32)
            for t in range(NTILE):
                oh_hi = ohp.tile([128, NHI], BF16)
                nc.vector.tensor_scalar(oh_hi[:], iota_hi[:], hi_f[:, t:t + 1], None,
                                        mybir.AluOpType.is_equal)
                oh_lo = ohp.tile([128, NLO], BF16)
                nc.vector.tensor_scalar(oh_lo[:], iota_lo[:], lo_f[:, t:t + 1], None,
                                        mybir.AluOpType.is_equal)
                nc.tensor.matmul(hf_psum[:], oh_hi[:], oh_lo[:],
                                 start=(t == 0), stop=(t == NTILE - 1))

            hf_sb = sm.tile([NHI, NLO], F32)
            nc.vector.tensor_copy(hf_sb[:], hf_psum[:])

            if stage == 1:
                nc.sync.dma_start(
                    out_dram.ap()[0:NF].rearrange("(a b) -> a b", a=NHI), hf_sb[:])
            else:
                # store padded fine hist to DRAM
                nc.sync.dma_start(hf_dram.ap()[0:PADL], z_row[:, 0:PADL])
                nc.sync.dma_start(hf_dram.ap()[PADL + NF:HF_LEN],
                                  z_row[:, 0:HF_LEN - PADL - NF])
                nc.sync.dma_start(
                    hf_dram.ap()[PADL:PADL + NF].rearrange("(a b) -> a b", a=NHI),
                    hf_sb[:])

                # ---------- allreduce over the 4 cores of this image ----------
                nc.gpsimd.collective_compute(
                    "AllReduce",
                    mybir.AluOpType.add,
                    ins=[hf_dram.ap().opt()],
                    outs=[hf_red.ap().opt()],
                    replica_groups=[[0, 1, 2, 3], [4, 5, 6, 7]],
                )

                if stage == 15:
                    hr_sb = sm.tile([NHI, NLO], F32)
                    nc.sync.dma_start(
                        hr_sb[:],
                        hf_red.ap()[PADL:PADL + NF].rearrange("(a b) -> a b", a=NHI))
                    nc.sync.dma_start(
                        out_dram.ap()[0:NF].rearrange("(a b) -> a b", a=NHI),
                        hr_sb[:])

                else:
                    # ---------- conv -> coarse hist (2 blocks of 128 bins) ----------
                    hist_cols = []
                    scr = sm.tile([128, TWIN], F32)
                    for blk in range(2):
                        hband = big.tile([128, TWIN], F32)
                        src = mk_ap(hf_red.ap(), 4096 * blk, [[32, 128], [1, TWIN]])
                        nc.sync.dma_start(hband[:], src)
                        hcol = sm.tile([128, 1], F32)
                        nc.vector.tensor_tensor(scr[:], hband[:], kw[:],
                                                mybir.AluOpType.mult)
                        nc.vector.tensor_reduce(hcol[:], scr[:],
                                                mybir.AxisListType.X,
                                                mybir.AluOpType.add)
                        hist_cols.append(hcol)

                    if stage == 17:
                        nc.sync.dma_start(
                            out_dram.ap()[0:128].rearrange("(a b) -> a b", a=128),
                            hist_cols[0][:])
                        nc.sync.dma_start(
                            out_dram.ap()[128:256].rearrange("(a b) -> a b", a=128),
                            hist_cols[1][:])

                    else:
                        # ---------- cdf via triangular matmul ----------
                        cdf_psum = psp.tile([1, N_BINS], F32)
                        nc.tensor.matmul(cdf_psum[:], hist_cols[0][:], tri0[:],
                                         start=True, stop=False)
                        nc.tensor.matmul(cdf_psum[:], hist_cols[1][:], tri1[:],
                                         start=False, stop=True)

                        if stage == 18:
                            cdump = sm.tile([1, N_BINS], F32)
                            nc.vector.tensor_copy(cdump[:], cdf_psum[:])
                            nc.sync.dma_start(
                                out_dram.ap()[0:N_BINS].rearrange("(a b) -> a b", a=1),
                                cdump[:])

                        else:
                            # cdfn = (cdf - cdf0) / (cdf_end - cdf0)
                            cdf_sb = sm.tile([1, N_BINS], F32)
                            nc.vector.tensor_copy(cdf_sb[:], cdf_psum[:])
                            c0 = cdf_sb[:, 0:1]
                            cend = cdf_sb[:, N_BINS - 1:N_BINS]
                            denom = sm.tile([1, 1], F32)
                            nc.vector.tensor_tensor(denom[:], cend, c0, mybir.AluOpType.subtract)
                            rden = sm.tile([1, 1], F32)
                            nc.vector.reciprocal(rden[:], denom[:])
                            cp_row = sm.tile([1, N_BINS + KWIN], F32)
                            nc.vector.memset(cp_row[:], 0.0)
                            nc.vector.tensor_scalar(cp_row[:, KWIN // 2:KWIN // 2 + N_BINS],
                                                    cdf_sb[:], c0, rden[:],
                                                    mybir.AluOpType.subtract, mybir.AluOpType.mult)
                            nc.sync.dma_start(cpad_dram.ap(), cp_row[:])

                            # ---------- G table build ----------
                            rhs_cv = sm.tile([KWIN, 2 * N_BINS], F32)
                            nc.sync.dma_start(rhs_cv[:, 0:N_BINS],
                                              mk_ap(cpad_dram.ap(), 0, [[1, KWIN], [1, N_BINS]]))
                            nc.sync.dma_start(rhs_cv[:, N_BINS:2 * N_BINS],
                                              mk_ap(vpad_dram.ap(), 0, [[1, KWIN], [1, N_BINS]]))
                            g_psum = psp.tile([MSUB, 2 * N_BINS], F32)
                            nc.tensor.matmul(g_psum[:], ww[:], rhs_cv[:], start=True, stop=True)
                            rec_den = sm.tile([MSUB, N_BINS], F32)
                            nc.vector.reciprocal(rec_den[:], g_psum[:, N_BINS:2 * N_BINS])
                            g_sb = sm.tile([MSUB, N_BINS], F32)
                            nc.vector.tensor_tensor(g_sb[:], g_psum[:, 0:N_BINS], rec_den[:],
                                                    mybir.AluOpType.mult)
                            # gtab[16*q' + m] = g_sb[m, q']
                            nc.sync.dma_start(
                                mk_ap(gtab_dram.ap(), 0, [[1, MSUB], [MSUB, N_BINS]]), g_sb[:])

                            if stage == 2:
                                nc.sync.dma_start(
                                    out_dram.ap()[0:TLEN].rearrange("(a b) -> a b", a=MSUB),
                                    g_sb[:].transpose([1, 0]) if False else g_sb[:])
                            else:
                                # ---------- broadcast table to all partitions ----------
                                tab_sb = big.tile([128, TLEN], F32)
                                nc.sync.dma_start(
                                    tab_sb[:],
                                    gtab_dram.ap().rearrange("(a b) -> a b", a=1)
                                    .to_broadcast((128, TLEN)))

                                # ---------- phase 2: per-pixel lookup ----------
                                # idx directly from the contiguous x tile;
                                # round(x*TGRID): HW convert rounds-to-nearest
                                idx_sb = big.tile([128, NTILE], I16)
                                nc.vector.tensor_scalar(idx_sb[:], x_sb[:],
                                                        float(TGRID), None,
                                                        mybir.AluOpType.mult)

                                gout = big.tile([128, NPX_GRP], F32)
                                nc.gpsimd.ap_gather(
                                    gout[:].rearrange("c (n d) -> c n d", d=1),
                                    tab_sb[:].rearrange("c (n d) -> c n d", d=1),
                                    idx_sb[:], channels=128, num_elems=TLEN, d=1,
                                    num_idxs=NPX_GRP)

                                # gout[16g, 16s+r] holds pixel 8192g+512r+s.
                                # Store as-is (8 contiguous descriptors); the
                                # host unshard undoes the 16x512 wrap.
                                nc.sync.dma_start(
                                    out_dram.ap().rearrange("(a b) -> a b", a=8),
                                    gout[::16, :])
    nc.compile()
    return nc


_NC_CACHE = None


def _get_nc():
    global _NC_CACHE
    if _NC_CACHE is None:
        _NC_CACHE = build_nc()
    return _NC_CACHE


def _axon_device_reset():
    """Recover a wedged axon terminal (NRT_EXEC_UNIT_UNRECOVERABLE)."""
    try:
        import ctypes
        import jax
        jax.devices()
        lib = ctypes.CDLL("/opt/axon/libaxon_pjrt.so")
        if hasattr(lib, "axon_reset"):
            lib.axon_reset.restype = ctypes.c_int64
            lib.axon_reset()
    except Exception:
        pass


def kernel(x: np.ndarray) -> np.ndarray:
    assert x.shape == (B, 1, H, W), x.shape
    x = np.ascontiguousarray(np.asarray(x, dtype=np.float32))
    nc = _get_nc()
    in_maps = []
    for core in range(N_CORES):
        b, q = core // 4, core % 4
        shard = x[b, 0, q * 128:(q + 1) * 128, :].reshape(QUARTER)
        in_maps.append({"x": np.ascontiguousarray(shard)})
    try:
        res = run_bass_kernel_spmd(nc, in_maps, core_ids=list(range(N_CORES)))
    except Exception:
        _axon_device_reset()
        res = run_bass_kernel_spmd(nc, in_maps, core_ids=list(range(N_CORES)))
    out = np.empty((B, 1, H, W), np.float32)
    for core in range(N_CORES):
        b, q = core // 4, core % 4
        r = res.results[core]["out"].reshape(8, 512, 16).transpose(0, 2, 1)
        out[b, 0, q * 128:(q + 1) * 128, :] = r.reshape(128, W)
    return out



# revision 6
# speedup vs baseline: 1.0073x; 1.0073x over previous
"""Trainium2 Bass kernel for nn_Equalize (soft histogram equalization).

Algorithm (per core; 8 cores, each owns a quarter of one of the 2 images):
  1. Fine histogram (8160 bins) of the core's 65536 pixels via two-level
     one-hot (96 x 85) outer-product matmuls accumulated in PSUM.
  2. AllReduce the fine histogram across the 4 cores of each image.
  3. Coarse 256-bin soft histogram = Toeplitz window-conv of the fine
     histogram with the Gaussian kernel (DVE mul+reduce on strided DMA views).
  4. cdf via triangular matmul; normalize to cdfn.
  5. G lookup table (4096 entries): G(v) = sum_j k(v-b_j)*cdfn[j] / sum_j k(v-b_j)
     computed with a small Toeplitz matmul (window of 32 bins).
  6. Per-pixel output = G[round(x*4080)] via GPSIMD ap_gather.

The output of the reference only depends on a pixel through the smooth 1-D
function G, so a fine table lookup reproduces it to ~1e-4.
"""
import os
import numpy as np

import concourse.bass as bass
import concourse.mybir as mybir
import concourse.tile as tile
import concourse.bacc as bacc
from concourse.bass_utils import run_bass_kernel_spmd

F32 = mybir.dt.float32
I32 = mybir.dt.int32
I16 = mybir.dt.int16
U16 = mybir.dt.uint16
BF16 = mybir.dt.bfloat16

B, H, W = 2, 512, 512
N_CORES = 8
QUARTER = H // 4 * W            # 65536 pixels per core
N_BINS = 256
TAU = 0.01
C = 1.0 / (2.0 * TAU * TAU)     # 5000
SQC = float(np.sqrt(C))
NF = 8160                        # fine-hist resolution (32*255)
NHI, NLO = 96, 85                # NF = NHI*NLO
TWIN = 1024                      # conv window (fine bins)
PADL = TWIN // 2                 # 512
HF_LEN = PADL + NF + (TWIN // 2 + 32)   # 9216, padded fine hist
TGRID = 4080                     # G-table grid (16*255)
MSUB = 16                        # table sub-samples per bin
TLEN = 4096                      # table allocation (num_elems)
KWIN = 32                        # G window in coarse bins
NTILE = QUARTER // 128           # 512 pixel tiles
NPX_GRP = QUARTER // 8           # 8192 pixels per gpsimd core group


def mk_ap(handle_ap, offset, pairs):
    import dataclasses
    return dataclasses.replace(handle_ap, offset=offset, ap=list(pairs))


def build_nc(stage=3):
    stage = int(os.environ.get("KERNEL_STAGE", stage))
    nc = bacc.Bacc()
    x_dram = nc.declare_dram_parameter("x", [QUARTER], F32, isOutput=False)
    out_dram = nc.declare_dram_parameter("out", [QUARTER], F32, isOutput=True)

    hf_dram = nc.dram_tensor("hf_local", [HF_LEN], F32)
    hf_red = nc.dram_tensor("hf_red", [HF_LEN], F32)
    cpad_dram = nc.dram_tensor("cpad", [N_BINS + KWIN], F32)    # 288
    vpad_dram = nc.dram_tensor("vpad", [N_BINS + KWIN], F32)
    gtab_dram = nc.dram_tensor("gtab", [TLEN], F32)

    with tile.TileContext(nc) as tc:
        with (
            tc.tile_pool(name="big", bufs=1) as big,
            tc.tile_pool(name="oh", bufs=4) as ohp,
            tc.tile_pool(name="small", bufs=1) as sm,
            tc.tile_pool(name="psum", bufs=1, space="PSUM") as psp,
        ):
            # ---------------- constants ----------------
            iota_hi_i = sm.tile([128, NHI], I32)
            nc.gpsimd.iota(iota_hi_i[:], pattern=[[1, NHI]], base=0, channel_multiplier=0)
            iota_hi = sm.tile([128, NHI], BF16)
            nc.vector.tensor_copy(iota_hi[:], iota_hi_i[:])

            iota_lo_i = sm.tile([128, NLO], I32)
            nc.gpsimd.iota(iota_lo_i[:], pattern=[[1, NLO]], base=0, channel_multiplier=0)
            iota_lo = sm.tile([128, NLO], BF16)
            nc.vector.tensor_copy(iota_lo[:], iota_lo_i[:])

            # kw[t] = exp(-C*((t-511.5)/NF)^2), replicated per partition
            kw_i = sm.tile([128, TWIN], I32)
            nc.gpsimd.iota(kw_i[:], pattern=[[1, TWIN]], base=0, channel_multiplier=0)
            kw_f = sm.tile([128, TWIN], F32)
            nc.vector.tensor_copy(kw_f[:], kw_i[:])
            kw_sq = sm.tile([128, TWIN], F32)
            bias_kw = sm.tile([128, 1], F32)
            nc.vector.memset(bias_kw[:], -SQC * (TWIN / 2 - 0.5) / NF)
            nc.scalar.activation(kw_sq[:], kw_f[:], mybir.ActivationFunctionType.Square,
                                 bias=bias_kw[:], scale=SQC / NF)
            kw = sm.tile([128, TWIN], F32)
            nc.scalar.activation(kw[:], kw_sq[:], mybir.ActivationFunctionType.Exp,
                                 scale=-1.0)

            # W_win lhsT [32 k, 16 m]: exp(-C*((m + 256 - 16k)/TGRID)^2)
            ww_i = sm.tile([KWIN, MSUB], I32)
            nc.gpsimd.iota(ww_i[:], pattern=[[1, MSUB]], base=256, channel_multiplier=-16)
            ww_f = sm.tile([KWIN, MSUB], F32)
            nc.vector.tensor_copy(ww_f[:], ww_i[:])
            ww_sq = sm.tile([KWIN, MSUB], F32)
            bias_z32 = sm.tile([KWIN, 1], F32)
            nc.vector.memset(bias_z32[:], 0.0)
            nc.scalar.activation(ww_sq[:], ww_f[:], mybir.ActivationFunctionType.Square,
                                 bias=bias_z32[:], scale=SQC / TGRID)
            ww = sm.tile([KWIN, MSUB], F32)
            nc.scalar.activation(ww[:], ww_sq[:], mybir.ActivationFunctionType.Exp,
                                 scale=-1.0)

            # triangular matrices for cumsum: iota j-k
            tri_i = sm.tile([128, N_BINS], I16)
            nc.gpsimd.iota(tri_i[:], pattern=[[1, N_BINS]], base=0, channel_multiplier=-1)
            tri0 = sm.tile([128, N_BINS], F32)
            nc.vector.tensor_scalar(tri0[:], tri_i[:], 0.0, None, mybir.AluOpType.is_ge)
            tri1 = sm.tile([128, N_BINS], F32)
            nc.vector.tensor_scalar(tri1[:], tri_i[:], 128.0, None, mybir.AluOpType.is_ge)

            # zero row for DRAM padding; ones/valid row
            z_row = sm.tile([1, PADL + TWIN // 2 + 32], F32)
            nc.vector.memset(z_row[:], 0.0)
            vp_row = sm.tile([1, N_BINS + KWIN], F32)
            nc.vector.memset(vp_row[:], 0.0)
            nc.vector.memset(vp_row[:, KWIN // 2:KWIN // 2 + N_BINS], 1.0)
            nc.sync.dma_start(vpad_dram.ap(), vp_row[:])

            # ---------------- phase 1: fine histogram ----------------
            x_sb = big.tile([128, NTILE], F32)
            nc.sync.dma_start(x_sb[:], x_dram.ap().rearrange("(p t) -> p t", p=128))

            # HW f32->int converts round-to-nearest-even, so floor(v) is
            # round(v - 0.5) (ties land on even, a half-fine-bin jitter).
            v_sb = big.tile([128, NTILE], F32)
            nc.vector.tensor_scalar(v_sb[:], x_sb[:], float(NHI), None, mybir.AluOpType.mult)
            hi_i = big.tile([128, NTILE], I32)
            nc.vector.tensor_scalar(hi_i[:], v_sb[:], 0.5, None, mybir.AluOpType.subtract)
            hi_f = big.tile([128, NTILE], F32)
            nc.vector.tensor_copy(hi_f[:], hi_i[:])
            fr_sb = big.tile([128, NTILE], F32)
            nc.vector.tensor_tensor(fr_sb[:], v_sb[:], hi_f[:], mybir.AluOpType.subtract)
            lo_i = big.tile([128, NTILE], I32)
            nc.vector.tensor_scalar(lo_i[:], fr_sb[:], float(NLO), 0.5,
                                    mybir.AluOpType.mult, mybir.AluOpType.subtract)
            lo_f = big.tile([128, NTILE], F32)
            nc.vector.tensor_copy(lo_f[:], lo_i[:])

            hf_psum = psp.tile([NHI, NLO], F32)
            for t in range(NTILE):
                oh_hi = ohp.tile([128, NHI], BF16)
                nc.vector.tensor_scalar(oh_hi[:], iota_hi[:], hi_f[:, t:t + 1], None,
                                        mybir.AluOpType.is_equal)
                oh_lo = ohp.tile([128, NLO], BF16)
                nc.vector.tensor_scalar(oh_lo[:], iota_lo[:], lo_f[:, t:t + 1], None,
                                        mybir.AluOpType.is_equal)
                nc.tensor.matmul(hf_psum[:], oh_hi[:], oh_lo[:],
                                 start=(t == 0), stop=(t == NTILE - 1))

            hf_sb = sm.tile([NHI, NLO], F32)
            nc.vector.tensor_copy(hf_sb[:], hf_psum[:])

            if stage == 1:
                nc.sync.dma_start(
                    out_dram.ap()[0:NF].rearrange("(a b) -> a b", a=NHI), hf_sb[:])
            else:
                # store padded fine hist to DRAM
                nc.sync.dma_start(hf_dram.ap()[0:PADL], z_row[:, 0:PADL])
                nc.sync.dma_start(hf_dram.ap()[PADL + NF:HF_LEN],
                                  z_row[:, 0:HF_LEN - PADL - NF])
                nc.sync.dma_start(
                    hf_dram.ap()[PADL:PADL + NF].rearrange("(a b) -> a b", a=NHI),
                    hf_sb[:])

                # ---------- allreduce over the 4 cores of this image ----------
                nc.gpsimd.collective_compute(
                    "AllReduce",
                    mybir.AluOpType.add,
                    ins=[hf_dram.ap().opt()],
                    outs=[hf_red.ap().opt()],
                    replica_groups=[[0, 1, 2, 3], [4, 5, 6, 7]],
                )

                if stage == 15:
                    hr_sb = sm.tile([NHI, NLO], F32)
                    nc.sync.dma_start(
                        hr_sb[:],
                        hf_red.ap()[PADL:PADL + NF].rearrange("(a b) -> a b", a=NHI))
                    nc.sync.dma_start(
                        out_dram.ap()[0:NF].rearrange("(a b) -> a b", a=NHI),
                        hr_sb[:])

                else:
                    # ---------- conv -> coarse hist (2 blocks of 128 bins) ----------
                    hist_cols = []
                    scr = sm.tile([128, TWIN], F32)
                    for blk in range(2):
                        hband = big.tile([128, TWIN], F32)
                        src = mk_ap(hf_red.ap(), 4096 * blk, [[32, 128], [1, TWIN]])
                        nc.sync.dma_start(hband[:], src)
                        hcol = sm.tile([128, 1], F32)
                        nc.vector.tensor_tensor(scr[:], hband[:], kw[:],
                                                mybir.AluOpType.mult)
                        nc.vector.tensor_reduce(hcol[:], scr[:],
                                                mybir.AxisListType.X,
                                                mybir.AluOpType.add)
                        hist_cols.append(hcol)

                    if stage == 17:
                        nc.sync.dma_start(
                            out_dram.ap()[0:128].rearrange("(a b) -> a b", a=128),
                            hist_cols[0][:])
                        nc.sync.dma_start(
                            out_dram.ap()[128:256].rearrange("(a b) -> a b", a=128),
                            hist_cols[1][:])

                    else:
                        # ---------- cdf via triangular matmul ----------
                        cdf_psum = psp.tile([1, N_BINS], F32)
                        nc.tensor.matmul(cdf_psum[:], hist_cols[0][:], tri0[:],
                                         start=True, stop=False)
                        nc.tensor.matmul(cdf_psum[:], hist_cols[1][:], tri1[:],
                                         start=False, stop=True)

                        if stage == 18:
                            cdump = sm.tile([1, N_BINS], F32)
                            nc.vector.tensor_copy(cdump[:], cdf_psum[:])
                            nc.sync.dma_start(
                                out_dram.ap()[0:N_BINS].rearrange("(a b) -> a b", a=1),
                                cdump[:])

                        else:
                            # cdfn = (cdf - cdf0) / (cdf_end - cdf0)
                            cdf_sb = sm.tile([1, N_BINS], F32)
                            nc.vector.tensor_copy(cdf_sb[:], cdf_psum[:])
                            c0 = cdf_sb[:, 0:1]
                            cend = cdf_sb[:, N_BINS - 1:N_BINS]
                            denom = sm.tile([1, 1], F32)
                            nc.vector.tensor_tensor(denom[:], cend, c0, mybir.AluOpType.subtract)
                            rden = sm.tile([1, 1], F32)
                            nc.vector.reciprocal(rden[:], denom[:])
                            cp_row = sm.tile([1, N_BINS + KWIN], F32)
                            nc.vector.memset(cp_row[:], 0.0)
                            nc.vector.tensor_scalar(cp_row[:, KWIN // 2:KWIN // 2 + N_BINS],
                                                    cdf_sb[:], c0, rden[:],
                                                    mybir.AluOpType.subtract, mybir.AluOpType.mult)
                            nc.sync.dma_start(cpad_dram.ap(), cp_row[:])

                            # ---------- G table build ----------
                            rhs_cv = sm.tile([KWIN, 2 * N_BINS], F32)
                            nc.sync.dma_start(rhs_cv[:, 0:N_BINS],
                                              mk_ap(cpad_dram.ap(), 0, [[1, KWIN], [1, N_BINS]]))
                            nc.sync.dma_start(rhs_cv[:, N_BINS:2 * N_BINS],
                                              mk_ap(vpad_dram.ap(), 0, [[1, KWIN], [1, N_BINS]]))
                            g_psum = psp.tile([MSUB, 2 * N_BINS], F32)
                            nc.tensor.matmul(g_psum[:], ww[:], rhs_cv[:], start=True, stop=True)
                            rec_den = sm.tile([MSUB, N_BINS], F32)
                            nc.vector.reciprocal(rec_den[:], g_psum[:, N_BINS:2 * N_BINS])
                            g_sb = sm.tile([MSUB, N_BINS], F32)
                            nc.vector.tensor_tensor(g_sb[:], g_psum[:, 0:N_BINS], rec_den[:],
                                                    mybir.AluOpType.mult)
                            # gtab[16*q' + m] = g_sb[m, q']
                            nc.sync.dma_start(
                                mk_ap(gtab_dram.ap(), 0, [[1, MSUB], [MSUB, N_BINS]]), g_sb[:])

                            if stage == 2:
                                nc.sync.dma_start(
                                    out_dram.ap()[0:TLEN].rearrange("(a b) -> a b", a=MSUB),
                                    g_sb[:].transpose([1, 0]) if False else g_sb[:])
                            else:
                                # ---------- broadcast table to all partitions ----------
                                tab_sb = big.tile([128, TLEN], F32)
                                nc.sync.dma_start(
                                    tab_sb[:],
                                    gtab_dram.ap().rearrange("(a b) -> a b", a=1)
                                    .to_broadcast((128, TLEN)))

                                # ---------- phase 2: per-pixel lookup ----------
                                # idx directly from the contiguous x tile;
                                # round(x*TGRID): HW convert rounds-to-nearest
                                idx_sb = big.tile([128, NTILE], U16)
                                nc.vector.tensor_scalar(idx_sb[:], x_sb[:],
                                                        float(TGRID), None,
                                                        mybir.AluOpType.mult)

                                # native Pool-engine gather: per 16-partition
                                # group, out[:, i] = tab[:, idx_i] with idx_i
                                # read column-major from the group's 16 rows.
                                # HW caps dst at 1024 elems/instruction.
                                gout = big.tile([128, NPX_GRP], F32)
                                for c in range(NTILE // 64):
                                    nc.gpsimd.indirect_copy(
                                        gout[:, 1024 * c:1024 * (c + 1)],
                                        tab_sb[:],
                                        idx_sb[:, 64 * c:64 * (c + 1)],
                                        i_know_ap_gather_is_preferred=True)

                                # gout[16g, 16s+r] holds pixel 8192g+512r+s.
                                # Store as-is (8 contiguous descriptors); the
                                # host unshard undoes the 16x512 wrap.
                                nc.sync.dma_start(
                                    out_dram.ap().rearrange("(a b) -> a b", a=8),
                                    gout[::16, :])
    nc.compile()
    return nc


_NC_CACHE = None


def _get_nc():
    global _NC_CACHE
    if _NC_CACHE is None:
        _NC_CACHE = build_nc()
    return _NC_CACHE


def _axon_device_reset():
    """Recover a wedged axon terminal (NRT_EXEC_UNIT_UNRECOVERABLE)."""
    try:
        import ctypes
        import jax
        jax.devices()
        lib = ctypes.CDLL("/opt/axon/libaxon_pjrt.so")
        if hasattr(lib, "axon_reset"):
            lib.axon_reset.restype = ctypes.c_int64
            lib.axon_reset()
    except Exception:
        pass


def kernel(x: np.ndarray) -> np.ndarray:
    assert x.shape == (B, 1, H, W), x.shape
    x = np.ascontiguousarray(np.asarray(x, dtype=np.float32))
    nc = _get_nc()
    in_maps = []
    for core in range(N_CORES):
        b, q = core // 4, core % 4
        shard = x[b, 0, q * 128:(q + 1) * 128, :].reshape(QUARTER)
        in_maps.append({"x": np.ascontiguousarray(shard)})
    try:
        res = run_bass_kernel_spmd(nc, in_maps, core_ids=list(range(N_CORES)))
    except Exception:
        _axon_device_reset()
        res = run_bass_kernel_spmd(nc, in_maps, core_ids=list(range(N_CORES)))
    out = np.empty((B, 1, H, W), np.float32)
    for core in range(N_CORES):
        b, q = core // 4, core % 4
        r = res.results[core]["out"].reshape(8, 8, 64, 16).transpose(0, 3, 1, 2)
        out[b, 0, q * 128:(q + 1) * 128, :] = r.reshape(128, W)
    return out



# revision 9
# speedup vs baseline: 1.9793x; 1.9651x over previous
"""Trainium2 Bass kernel for nn_Equalize (soft histogram equalization).

Per core (8 cores, each owns a quarter of one of the 2 images):
  1. 256-bin histogram of round(x*255) via one-hot (DVE is_equal) +
     ones-lhsT matmul contraction (PE), 2 pixel columns per matmul.
  2. AllReduce the histogram across the 4 cores of each image.
  3. Gaussian re-smoothing of the histogram (Toeplitz matmuls) to match
     the reference KDE, then cdf via triangular matmuls.
  4. G evaluated at 65 knots m/64: G(t) = sum_j k(t-b_j) cdf_j / sum_j k,
     via small matmuls with derivative_erf-built Gaussian weights,
     normalized to cdfn afterwards (affine in cdf).
  5. Per-pixel output = piecewise-linear interp of G evaluated with a
     relu expansion: out = G0 + beta0 x + sum_m w_m relu(x - m/64).
     Knot terms run on the Scalar engine as relu(w~ x - w~ th) with
     w~ = w + S shifted positive; the shift is corrected exactly with
     S*Q(x), Q = k x - (k^2+k)/128, k = floor(64 x).  Terms accumulate
     in an SBUF stack reduced on DVE.  No gpsimd gather anywhere.
"""
import os
import numpy as np

import concourse.bass as bass
import concourse.mybir as mybir
import concourse.tile as tile
import concourse.bacc as bacc
from concourse.bass_utils import run_bass_kernel_spmd

F32 = mybir.dt.float32
I32 = mybir.dt.int32
I16 = mybir.dt.int16
BF16 = mybir.dt.bfloat16

B, H, W = 2, 512, 512
N_CORES = 8
QUARTER = H // 4 * W            # 65536 pixels per core
NCOL = QUARTER // 128           # 512 pixel columns
NB = 256                        # histogram bins (reference bins j/255)
TAU = 0.01
SQC = float(np.sqrt(1.0 / (2.0 * TAU * TAU)))   # 70.71
NK = 65                         # PWL knots at m/64, m=0..64
NSEG = 64
SHIFT = 2.0                     # relu weight positivity shift
NSLOT = 66                      # 63 knots + linear + 2 Q-correction slots
AF = mybir.ActivationFunctionType


def mk_ap(handle_ap, offset, pairs):
    import dataclasses
    return dataclasses.replace(handle_ap, offset=offset, ap=list(pairs))


def build_nc(stage=3):
    stage = int(os.environ.get("KERNEL_STAGE", stage))
    nc = bacc.Bacc()
    x_dram = nc.declare_dram_parameter("x", [QUARTER], F32, isOutput=False)
    out_dram = nc.declare_dram_parameter("out", [QUARTER], F32, isOutput=True)

    hrows_dram = nc.dram_tensor("hrows", [2 * NB], F32)
    hred_dram = nc.dram_tensor("hred", [2 * NB], F32)

    with tile.TileContext(nc) as tc:
        with (
            tc.tile_pool(name="big", bufs=1) as big,
            tc.tile_pool(name="oh", bufs=4) as ohp,
            tc.tile_pool(name="small", bufs=1) as sm,
            tc.tile_pool(name="psum", bufs=1, space="PSUM") as psp,
        ):
            # ---------------- constants ----------------
            iota_i = sm.tile([128, NB], I32)
            nc.gpsimd.iota(iota_i[:], pattern=[[1, NB]], base=0, channel_multiplier=0)
            iotaB = sm.tile([128, NB], BF16)
            nc.vector.tensor_copy(iotaB[:], iota_i[:])

            ones_col = sm.tile([128, 1], BF16)
            nc.vector.memset(ones_col[:], 1.0)
            onesf_col = sm.tile([128, 1], F32)
            nc.vector.memset(onesf_col[:], 1.0)
            ones_row = sm.tile([1, 128], F32)
            nc.vector.memset(ones_row[:], 1.0)
            ones_sq = sm.tile([128, 128], F32)
            nc.vector.memset(ones_sq[:], 1.0)

            # triangular: tri0[p, j] = 1 if j >= p ; tri1[p, j] = 1 if j >= p+128
            tri_i = sm.tile([128, NB], I16)
            nc.gpsimd.iota(tri_i[:], pattern=[[1, NB]], base=0, channel_multiplier=-1)
            tri0 = sm.tile([128, NB], F32)
            nc.vector.tensor_scalar(tri0[:], tri_i[:], 0.0, None, mybir.AluOpType.is_ge)
            tri1 = sm.tile([128, NB], F32)
            nc.vector.tensor_scalar(tri1[:], tri_i[:], 128.0, None, mybir.AluOpType.is_ge)

            # Gaussian tiles via derivative_erf(z) = 2/sqrt(pi) exp(-z^2)
            # (the 2/sqrt(pi) factor cancels in every normalization)
            def gauss_tile(npart, nfree, base, ch_mult, step, scale):
                ti = sm.tile([npart, nfree], I32)
                nc.gpsimd.iota(ti[:], pattern=[[step, nfree]], base=base,
                               channel_multiplier=ch_mult)
                tf = sm.tile([npart, nfree], F32)
                nc.vector.tensor_copy(tf[:], ti[:])
                tg = sm.tile([npart, nfree], F32)
                nc.scalar.activation(tg[:], tf[:], AF.Derivative_Erf, scale=scale)
                return tg

            # Toeplitz blocks: kt_a[p, jj] = g(p - jj), jj = 0..255
            kt_a = gauss_tile(128, NB, 0, 1, -1, SQC / 255.0)
            # kt_b[p, j] = g(p + 128 - j), j = 0..127
            kt_b = gauss_tile(128, 128, 128, 1, -1, SQC / 255.0)
            # knot weights: wt[p, m] = k(theta_m - b_p) via 255 m - 64 p (x16320)
            wt0 = gauss_tile(128, NK, 0, -64, 255, SQC / 16320.0)
            wt1 = gauss_tile(128, NK, -64 * 128, -64, 255, SQC / 16320.0)

            # knot position row [1, NK]: theta_m = m / 64
            th_i = sm.tile([1, NK], I32)
            nc.gpsimd.iota(th_i[:], pattern=[[1, NK]], base=0, channel_multiplier=0)
            th_row = sm.tile([1, NK], F32)
            nc.vector.tensor_scalar(th_row[:], th_i[:], 1.0 / NSEG, None,
                                    mybir.AluOpType.mult)

            # ---------------- phase 0: prep ----------------
            x_sb = big.tile([128, NCOL], F32)
            nc.sync.dma_start(x_sb[:], x_dram.ap().rearrange("(p t) -> p t", p=128))

            qi = big.tile([128, NCOL], I32)
            nc.vector.tensor_scalar(qi[:], x_sb[:], 255.0, None, mybir.AluOpType.mult)
            qf = big.tile([128, NCOL], F32)
            nc.vector.tensor_copy(qf[:], qi[:])

            # phase-2 prep (independent of histogram; runs early on DVE)
            u_sb = big.tile([128, NCOL], F32)
            nc.vector.tensor_scalar(u_sb[:], x_sb[:], float(NSEG), 0.5,
                                    mybir.AluOpType.mult, mybir.AluOpType.subtract)
            ki = big.tile([128, NCOL], I32)
            nc.vector.tensor_copy(ki[:], u_sb[:])
            kf = big.tile([128, NCOL], F32)
            nc.vector.tensor_copy(kf[:], ki[:])
            kx = big.tile([128, NCOL], F32)
            nc.vector.tensor_tensor(kx[:], kf[:], x_sb[:], mybir.AluOpType.mult)
            k2k = big.tile([128, NCOL], F32)
            nc.vector.tensor_tensor(k2k[:], kf[:], kf[:], mybir.AluOpType.mult)
            nc.vector.tensor_tensor(k2k[:], k2k[:], kf[:], mybir.AluOpType.add)

            # ---------------- phase 1: histogram ----------------
            hist_psum = psp.tile([1, 2 * NB], F32)
            for c in range(0, NCOL, 2):
                oh = ohp.tile([128, 2 * NB], BF16)
                nc.vector.tensor_scalar(oh[:, 0:NB], iotaB[:], qf[:, c:c + 1],
                                        None, mybir.AluOpType.is_equal)
                nc.vector.tensor_scalar(oh[:, NB:2 * NB], iotaB[:], qf[:, c + 1:c + 2],
                                        None, mybir.AluOpType.is_equal)
                nc.tensor.matmul(hist_psum[:], ones_col[:], oh[:],
                                 start=(c == 0), stop=(c == NCOL - 2))

            hist_row = sm.tile([1, 2 * NB], F32)
            nc.vector.tensor_copy(hist_row[:], hist_psum[:])
            rows2 = sm.tile([1, 2 * NB], F32)
            nc.vector.memset(rows2[:], 0.0)
            nc.vector.tensor_tensor(rows2[:, 0:NB], hist_row[:, 0:NB],
                                    hist_row[:, NB:2 * NB], mybir.AluOpType.add)
            nc.sync.dma_start(hrows_dram.ap(), rows2[:])

            if stage == 1:
                nc.sync.dma_start(
                    out_dram.ap()[0:2 * NB].rearrange("(a b) -> a b", a=1),
                    rows2[:])
            else:
                # ---------- allreduce over the 4 cores of this image ----------
                nc.gpsimd.collective_compute(
                    "AllReduce",
                    mybir.AluOpType.add,
                    ins=[hrows_dram.ap().opt()],
                    outs=[hred_dram.ap().opt()],
                    replica_groups=[[0, 1, 2, 3], [4, 5, 6, 7]],
                )

                ncol_all = sm.tile([128, 2], F32)
                nc.sync.dma_start(ncol_all[:],
                                  mk_ap(hred_dram.ap(), 0, [[1, 128], [128, 2]]))

                # ---------- smooth hist (Toeplitz) -> hist_col [128, 2] ----------
                histc_psum = psp.tile([128, 2], F32)
                nc.tensor.matmul(histc_psum[:], kt_a[:, 0:128], ncol_all[:, 0:2],
                                 start=True, stop=False)
                nc.tensor.matmul(histc_psum[:, 1:2], kt_a[:, 128:256],
                                 ncol_all[:, 0:1], start=False, stop=False)
                nc.tensor.matmul(histc_psum[:, 0:1], kt_b[:], ncol_all[:, 1:2],
                                 start=False, stop=True)
                hist_col = sm.tile([128, 2], F32)
                nc.vector.tensor_copy(hist_col[:], histc_psum[:])

                # ---------- cdf column [128, 2] and row [1, 256] ----------
                zh = sm.tile([128, 2], F32)
                nc.vector.memset(zh[:], 0.0)
                nc.vector.tensor_copy(zh[:, 1:2], hist_col[:, 0:1])
                cdfc_psum = psp.tile([128, 2], F32)
                nc.tensor.matmul(cdfc_psum[:], tri0[:, 0:128], hist_col[:],
                                 start=True, stop=False)
                nc.tensor.matmul(cdfc_psum[:], ones_sq[:], zh[:],
                                 start=False, stop=True)
                cdf_col = sm.tile([128, 2], F32)
                nc.vector.tensor_copy(cdf_col[:], cdfc_psum[:])

                cdfr_psum = psp.tile([1, NB], F32)
                nc.tensor.matmul(cdfr_psum[:], hist_col[:, 0:1], tri0[:],
                                 start=True, stop=False)
                nc.tensor.matmul(cdfr_psum[:], hist_col[:, 1:2], tri1[:],
                                 start=False, stop=True)
                cdf_row = sm.tile([1, NB], F32)
                nc.vector.tensor_copy(cdf_row[:], cdfr_psum[:])

                # ---------- G at knots ----------
                num_psum = psp.tile([1, NK], F32)
                nc.tensor.matmul(num_psum[:], cdf_col[:, 0:1], wt0[:],
                                 start=True, stop=False)
                nc.tensor.matmul(num_psum[:], cdf_col[:, 1:2], wt1[:],
                                 start=False, stop=True)
                den_psum = psp.tile([1, NK], F32)
                nc.tensor.matmul(den_psum[:], onesf_col[:], wt0[:],
                                 start=True, stop=False)
                nc.tensor.matmul(den_psum[:], onesf_col[:], wt1[:],
                                 start=False, stop=True)

                rden = sm.tile([1, NK], F32)
                nc.vector.reciprocal(rden[:], den_psum[:])
                g_raw = sm.tile([1, NK], F32)
                nc.vector.tensor_tensor(g_raw[:], num_psum[:], rden[:],
                                        mybir.AluOpType.mult)
                # normalize: G = (g_raw - c0) / (cend - c0)
                c0 = cdf_row[:, 0:1]
                dnorm = sm.tile([1, 1], F32)
                nc.vector.tensor_tensor(dnorm[:], cdf_row[:, NB - 1:NB], c0,
                                        mybir.AluOpType.subtract)
                rnorm = sm.tile([1, 1], F32)
                nc.vector.reciprocal(rnorm[:], dnorm[:])
                g_row = sm.tile([1, NK], F32)
                nc.vector.tensor_scalar(g_row[:], g_raw[:], c0, rnorm[:],
                                        mybir.AluOpType.subtract,
                                        mybir.AluOpType.mult)

                # ---------- PWL coefficients ----------
                beta = sm.tile([1, NSEG], F32)
                nc.vector.tensor_tensor(beta[:], g_row[:, 1:NK], g_row[:, 0:NSEG],
                                        mybir.AluOpType.subtract)
                nc.vector.tensor_scalar(beta[:], beta[:], float(NSEG), None,
                                        mybir.AluOpType.mult)
                wsh = sm.tile([1, NSEG - 1], F32)   # w~_m = w_m + S, m = 1..63
                nc.vector.tensor_tensor(wsh[:], beta[:, 1:NSEG],
                                        beta[:, 0:NSEG - 1],
                                        mybir.AluOpType.subtract)
                nc.vector.tensor_scalar(wsh[:], wsh[:], SHIFT, None,
                                        mybir.AluOpType.add)

                # coef row: [0:63] w~ ; [63:126] -w~*theta ; [126] beta0 ; [127] G0
                coef_row = sm.tile([1, 128], F32)
                nc.vector.memset(coef_row[:], 0.0)
                nc.vector.tensor_copy(coef_row[:, 0:NSEG - 1], wsh[:])
                nc.vector.tensor_tensor(coef_row[:, 63:63 + NSEG - 1], wsh[:],
                                        th_row[:, 1:NSEG], mybir.AluOpType.mult)
                nc.vector.tensor_scalar(coef_row[:, 63:63 + NSEG - 1],
                                        coef_row[:, 63:63 + NSEG - 1], -1.0,
                                        None, mybir.AluOpType.mult)
                nc.vector.tensor_copy(coef_row[:, 126:127], beta[:, 0:1])
                nc.vector.tensor_copy(coef_row[:, 127:128], g_row[:, 0:1])

                coef_psum = psp.tile([128, 128], F32)
                nc.tensor.matmul(coef_psum[:], ones_row[:], coef_row[:],
                                 start=True, stop=True)
                coef = sm.tile([128, 128], F32)
                nc.vector.tensor_copy(coef[:], coef_psum[:])

                if stage == 2:
                    nc.sync.dma_start(
                        out_dram.ap()[0:128].rearrange("(a b) -> a b", a=1),
                        coef_row[:])
                else:
                    # ---------------- phase 2: PWL evaluation ----------------
                    rbuf = big.tile([128, NSLOT * NCOL], F32)

                    def slot(m):
                        return rbuf[:, m * NCOL:(m + 1) * NCOL]

                    # knot slots 0..62 on scalar engine: relu(w~ x - w~ theta)
                    for m in range(1, NSEG):
                        nc.scalar.activation(slot(m - 1), x_sb[:], AF.Relu,
                                             bias=coef[:, 63 + m - 1:63 + m],
                                             scale=coef[:, m - 1:m])
                    # linear slot: beta0 x + G0
                    nc.vector.tensor_scalar(slot(NSEG - 1), x_sb[:],
                                            coef[:, 126:127], coef[:, 127:128],
                                            mybir.AluOpType.mult,
                                            mybir.AluOpType.add)
                    # Q correction: -S*(k x) + (S/128)*(k^2+k)
                    nc.vector.tensor_scalar(slot(NSEG), kx[:], -SHIFT, None,
                                            mybir.AluOpType.mult)
                    nc.vector.tensor_scalar(slot(NSEG + 1), k2k[:],
                                            SHIFT / 128.0, None,
                                            mybir.AluOpType.mult)

                    # reduce over slots in two halves (overlaps scalar engine)
                    half = NSLOT // 2
                    r3a = rbuf[:, 0:half * NCOL].rearrange(
                        "p (m t) -> p t m", m=half)
                    r3b = rbuf[:, half * NCOL:NSLOT * NCOL].rearrange(
                        "p (m t) -> p t m", m=NSLOT - half)
                    red0 = big.tile([128, NCOL], F32)
                    nc.vector.tensor_reduce(red0[:], r3a, mybir.AxisListType.X,
                                            mybir.AluOpType.add)
                    red1 = big.tile([128, NCOL], F32)
                    nc.vector.tensor_reduce(red1[:], r3b, mybir.AxisListType.X,
                                            mybir.AluOpType.add)
                    out_sb = big.tile([128, NCOL], F32)
                    nc.vector.tensor_tensor(out_sb[:], red0[:], red1[:],
                                            mybir.AluOpType.add)

                    nc.sync.dma_start(
                        out_dram.ap().rearrange("(p t) -> p t", p=128),
                        out_sb[:])
    nc.compile()
    return nc


_NC_CACHE = None


def _get_nc():
    global _NC_CACHE
    if _NC_CACHE is None:
        _NC_CACHE = build_nc()
    return _NC_CACHE


def _axon_device_reset():
    """Recover a wedged axon terminal (NRT_EXEC_UNIT_UNRECOVERABLE)."""
    try:
        import ctypes
        import jax
        jax.devices()
        lib = ctypes.CDLL("/opt/axon/libaxon_pjrt.so")
        if hasattr(lib, "axon_reset"):
            lib.axon_reset.restype = ctypes.c_int64
            lib.axon_reset()
    except Exception:
        pass


def kernel(x: np.ndarray) -> np.ndarray:
    assert x.shape == (B, 1, H, W), x.shape
    x = np.ascontiguousarray(np.asarray(x, dtype=np.float32))
    nc = _get_nc()
    in_maps = []
    for core in range(N_CORES):
        b, q = core // 4, core % 4
        shard = x[b, 0, q * 128:(q + 1) * 128, :].reshape(QUARTER)
        in_maps.append({"x": np.ascontiguousarray(shard)})
    try:
        res = run_bass_kernel_spmd(nc, in_maps, core_ids=list(range(N_CORES)))
    except Exception:
        _axon_device_reset()
        res = run_bass_kernel_spmd(nc, in_maps, core_ids=list(range(N_CORES)))
    out = np.empty((B, 1, H, W), np.float32)
    for core in range(N_CORES):
        b, q = core // 4, core % 4
        out[b, 0, q * 128:(q + 1) * 128, :] = \
            res.results[core]["out"].reshape(128, W)
    return out


# revision 13
# speedup vs baseline: 2.7240x; 1.3762x over previous
"""Trainium2 Bass kernel for nn_Equalize (soft histogram equalization).

Per core (8 cores, each owns a quarter of one of the 2 images):
  1. Histogram: most pixel columns one-hot binned to a 128-level fine
     grid (DVE is_equal, bf16) and contracted with a ones-lhsT matmul
     (PE); NACT columns instead evaluate the exact Gaussian KDE on the
     256-bin reference grid on the Scalar engine (derivative_erf).
  2. AllReduce both partial histograms across the 4 cores of each image.
  3. The fine one-hot histogram is smoothed onto the 256-bin grid with a
     Gaussian Toeplitz matmul and added to the KDE part -> reference
     khist; cdf via triangular matmuls.
  4. G at 49 knots m/48: G(t) = sum_j k(t-b_j) cdf_j / sum_j k(t-b_j),
     via Gaussian-weight matmuls; normalized to cdfn afterwards.
  5. Per-pixel output = PWL interp of G via a relu expansion evaluated
     on Scalar (relu(w~ x - w~ th)) and DVE (max(w~ x, w~ th)) with
     w~ = w + S > 0; the shift is removed exactly with the closed form
     S*(k x - (k^2+k)/96), k = floor(48 x).  Terms accumulate into a
     running DVE sum.  No gpsimd gather anywhere.
"""
import os
import numpy as np

import concourse.bass as bass
import concourse.mybir as mybir
import concourse.tile as tile
import concourse.bacc as bacc
from concourse.bass_utils import run_bass_kernel_spmd

F32 = mybir.dt.float32
I32 = mybir.dt.int32
I16 = mybir.dt.int16
BF16 = mybir.dt.bfloat16

B, H, W = 2, 512, 512
N_CORES = 8
QUARTER = H // 4 * W            # 65536 pixels per core
NCOL = QUARTER // 128           # 512 pixel columns
NB = 256                        # reference histogram bins j/255
NFB = 128                       # fine one-hot grid p/127
TAU = 0.01
SQC = float(np.sqrt(1.0 / (2.0 * TAU * TAU)))   # 70.71
NSEG = 48                       # PWL segments, knots at m/48
NK = NSEG + 1
ND = 4                          # knots evaluated on DVE (m = NSEG-ND..NSEG-1)
SHIFT = 2.0                     # relu weight positivity shift
NACT = int(os.environ.get("KERNEL_NACT", 128))   # columns on scalar engine
AF = mybir.ActivationFunctionType


def mk_ap(handle_ap, offset, pairs):
    import dataclasses
    return dataclasses.replace(handle_ap, offset=offset, ap=list(pairs))


def build_nc(stage=3):
    stage = int(os.environ.get("KERNEL_STAGE", stage))
    nc = bacc.Bacc()
    x_dram = nc.declare_dram_parameter("x", [QUARTER], F32, isOutput=False)
    out_dram = nc.declare_dram_parameter("out", [QUARTER], F32, isOutput=True)

    NRED = NFB + NB             # 384: fine one-hot row + kde row
    hrows_dram = nc.dram_tensor("hrows", [NRED], F32)
    hred_dram = nc.dram_tensor("hred", [NRED], F32)

    NOH = NCOL - NACT           # one-hot columns

    with tile.TileContext(nc) as tc:
        with (
            tc.tile_pool(name="big", bufs=1) as big,
            tc.tile_pool(name="oh", bufs=4) as ohp,
            tc.tile_pool(name="term", bufs=4) as tp,
            tc.tile_pool(name="small", bufs=1) as sm,
            tc.tile_pool(name="psum", bufs=1, space="PSUM") as psp,
        ):
            # ---------------- constants ----------------
            iota_i = sm.tile([128, NFB], I32)
            nc.gpsimd.iota(iota_i[:], pattern=[[1, NFB]], base=0, channel_multiplier=0)
            iotaB = sm.tile([128, NFB], BF16)
            nc.vector.tensor_copy(iotaB[:], iota_i[:])
            iota256_i = sm.tile([128, NB], I32)
            nc.gpsimd.iota(iota256_i[:], pattern=[[1, NB]], base=0,
                           channel_multiplier=0)
            iota256 = sm.tile([128, NB], F32)
            nc.vector.tensor_copy(iota256[:], iota256_i[:])

            ones_col = sm.tile([128, 1], BF16)
            nc.vector.memset(ones_col[:], 1.0)
            onesf_col = sm.tile([128, 1], F32)
            nc.vector.memset(onesf_col[:], 1.0)
            ones_row = sm.tile([1, 128], F32)
            nc.vector.memset(ones_row[:], 1.0)

            # triangular: tri0[p, j] = 1 if j >= p ; tri1: j >= p+128
            tri_i = sm.tile([128, NB], I16)
            nc.gpsimd.iota(tri_i[:], pattern=[[1, NB]], base=0, channel_multiplier=-1)
            tri0 = sm.tile([128, NB], F32)
            nc.vector.tensor_scalar(tri0[:], tri_i[:], 0.0, None, mybir.AluOpType.is_ge)
            tri1 = sm.tile([128, NB], F32)
            nc.vector.tensor_scalar(tri1[:], tri_i[:], 128.0, None, mybir.AluOpType.is_ge)

            identity = sm.tile([128, 128], F32)
            id_i = sm.tile([128, 128], I16)
            nc.gpsimd.iota(id_i[:], pattern=[[1, 128]], base=0, channel_multiplier=-1)
            nc.vector.tensor_scalar(identity[:], id_i[:], 0.0, None,
                                    mybir.AluOpType.is_equal)

            # Gaussian tiles via derivative_erf(z) = 2/sqrt(pi) exp(-z^2);
            # the 2/sqrt(pi) factor is common to Toeplitz, KDE and knot
            # weights, so it cancels in all normalizations.
            def gauss_tile(npart, nfree, base, ch_mult, step, scale):
                ti = sm.tile([npart, nfree], I32)
                nc.gpsimd.iota(ti[:], pattern=[[step, nfree]], base=base,
                               channel_multiplier=ch_mult)
                tf = sm.tile([npart, nfree], F32)
                nc.vector.tensor_copy(tf[:], ti[:])
                tg = sm.tile([npart, nfree], F32)
                nc.scalar.activation(tg[:], tf[:], AF.Derivative_Erf, scale=scale)
                return tg

            # Toeplitz fine->coarse: KT[p, j] = k(p/127 - j/255), x32385
            ktoep = gauss_tile(128, NB, 0, 255, -127, SQC / 32385.0)
            # knot weights: wt[p, m] = k(m/48 - b_p) via 255 m - 48 j (x12240)
            wt0 = gauss_tile(128, NK, 0, -48, 255, SQC / 12240.0)
            wt1 = gauss_tile(128, NK, -48 * 128, -48, 255, SQC / 12240.0)

            # knot position row [1, NK]: theta_m = m / 48
            th_i = sm.tile([1, NK], I32)
            nc.gpsimd.iota(th_i[:], pattern=[[1, NK]], base=0, channel_multiplier=0)
            th_row = sm.tile([1, NK], F32)
            nc.vector.tensor_scalar(th_row[:], th_i[:], 1.0 / NSEG, None,
                                    mybir.AluOpType.mult)

            # ---------------- phase 0: prep ----------------
            x_sb = big.tile([128, NCOL], F32)
            nc.sync.dma_start(x_sb[:], x_dram.ap().rearrange("(p t) -> p t", p=128))

            qi = big.tile([128, NCOL], I32)
            nc.vector.tensor_scalar(qi[:], x_sb[:], float(NFB - 1), None,
                                    mybir.AluOpType.mult)
            qf = big.tile([128, NCOL], F32)
            nc.vector.tensor_copy(qf[:], qi[:])
            xs = big.tile([128, NCOL], F32)
            nc.vector.tensor_scalar(xs[:], x_sb[:], -SQC, None, mybir.AluOpType.mult)

            # phase-2 prep + Q-correction accumulator init (needs only x)
            u_sb = big.tile([128, NCOL], F32)
            nc.vector.tensor_scalar(u_sb[:], x_sb[:], float(NSEG), 0.5,
                                    mybir.AluOpType.mult, mybir.AluOpType.subtract)
            ki = big.tile([128, NCOL], I32)
            nc.vector.tensor_copy(ki[:], u_sb[:])
            kf = big.tile([128, NCOL], F32)
            nc.vector.tensor_copy(kf[:], ki[:])
            kx = big.tile([128, NCOL], F32)
            nc.vector.tensor_tensor(kx[:], kf[:], x_sb[:], mybir.AluOpType.mult)
            k2k = big.tile([128, NCOL], F32)
            nc.vector.tensor_tensor(k2k[:], kf[:], kf[:], mybir.AluOpType.mult)
            nc.vector.tensor_tensor(k2k[:], k2k[:], kf[:], mybir.AluOpType.add)
            acc = big.tile([128, NCOL], F32)
            nc.vector.tensor_scalar(acc[:], kx[:], -SHIFT, None, mybir.AluOpType.mult)
            qc2 = big.tile([128, NCOL], F32)
            nc.vector.tensor_scalar(qc2[:], k2k[:], SHIFT / (2.0 * NSEG), None,
                                    mybir.AluOpType.mult)
            nc.vector.tensor_tensor(acc[:], acc[:], qc2[:], mybir.AluOpType.add)

            # ---------------- phase 1: histograms ----------------
            # one-hot columns -> fine 128-bin psum row [1, 256] (2 cols/mm)
            oh_psum = psp.tile([1, 2 * NFB], F32)
            for c in range(0, NOH, 2):
                oh = ohp.tile([128, 2 * NFB], BF16)
                nc.vector.tensor_scalar(oh[:, 0:NFB], iotaB[:], qf[:, c:c + 1],
                                        None, mybir.AluOpType.is_equal)
                nc.vector.tensor_scalar(oh[:, NFB:2 * NFB], iotaB[:],
                                        qf[:, c + 1:c + 2],
                                        None, mybir.AluOpType.is_equal)
                nc.tensor.matmul(oh_psum[:], ones_col[:], oh[:],
                                 start=(c == 0), stop=(c == NOH - 2))

            # KDE columns -> coarse 256-bin psum row [1, 512] (2 cols/mm)
            if NACT > 0:
                kde_psum = psp.tile([1, 2 * NB], F32)
                for ci in range(0, NACT, 2):
                    kt = ohp.tile([128, 2 * NB], BF16)
                    c = NOH + ci
                    nc.scalar.activation(kt[:, 0:NB], iota256[:], AF.Derivative_Erf,
                                         bias=xs[:, c:c + 1], scale=SQC / 255.0)
                    nc.scalar.activation(kt[:, NB:2 * NB], iota256[:],
                                         AF.Derivative_Erf,
                                         bias=xs[:, c + 1:c + 2], scale=SQC / 255.0)
                    nc.tensor.matmul(kde_psum[:], ones_col[:], kt[:],
                                     start=(ci == 0), stop=(ci == NACT - 2))

            hrow = sm.tile([1, 2 * NB], F32)
            nc.vector.tensor_copy(hrow[:, 0:2 * NFB], oh_psum[:])
            rows = sm.tile([1, NRED], F32)
            nc.vector.tensor_tensor(rows[:, 0:NFB], hrow[:, 0:NFB],
                                    hrow[:, NFB:2 * NFB], mybir.AluOpType.add)
            if NACT > 0:
                nc.vector.tensor_copy(hrow[:], kde_psum[:])
                nc.vector.tensor_tensor(rows[:, NFB:NFB + NB], hrow[:, 0:NB],
                                        hrow[:, NB:2 * NB], mybir.AluOpType.add)
            else:
                nc.vector.memset(rows[:, NFB:NFB + NB], 0.0)
            nc.sync.dma_start(hrows_dram.ap(), rows[:])

            if stage == 1:
                nc.sync.dma_start(
                    out_dram.ap()[0:NRED].rearrange("(a b) -> a b", a=1), rows[:])
            else:
                # ---------- allreduce over the 4 cores of this image ----------
                nc.gpsimd.collective_compute(
                    "AllReduce",
                    mybir.AluOpType.add,
                    ins=[hrows_dram.ap().opt()],
                    outs=[hred_dram.ap().opt()],
                    replica_groups=[[0, 1, 2, 3], [4, 5, 6, 7]],
                )

                ohq_col = sm.tile([128, 1], F32)
                nc.sync.dma_start(ohq_col[:],
                                  mk_ap(hred_dram.ap(), 0, [[1, 128], [128, 1]]))
                kde_col = sm.tile([128, 2], F32)
                nc.sync.dma_start(kde_col[:],
                                  mk_ap(hred_dram.ap(), NFB, [[1, 128], [128, 2]]))

                # ---------- khist_col [128, 2] = KT @ onehot + kde ----------
                histc_psum = psp.tile([128, 2], F32)
                nc.tensor.matmul(histc_psum[:], identity[:], kde_col[:],
                                 start=True, stop=False)
                nc.tensor.matmul(histc_psum[:, 0:1], ktoep[:, 0:128], ohq_col[:],
                                 start=False, stop=False)
                nc.tensor.matmul(histc_psum[:, 1:2], ktoep[:, 128:256], ohq_col[:],
                                 start=False, stop=True)
                hist_col = sm.tile([128, 2], F32)
                nc.vector.tensor_copy(hist_col[:], histc_psum[:])

                # ---------- cdf column [128, 2] and row [1, 256] ----------
                zh = sm.tile([128, 2], F32)
                nc.vector.memset(zh[:], 0.0)
                nc.vector.tensor_copy(zh[:, 1:2], hist_col[:, 0:1])
                ones_sq = sm.tile([128, 128], F32)
                nc.vector.memset(ones_sq[:], 1.0)
                cdfc_psum = psp.tile([128, 2], F32)
                nc.tensor.matmul(cdfc_psum[:], tri0[:, 0:128], hist_col[:],
                                 start=True, stop=False)
                nc.tensor.matmul(cdfc_psum[:], ones_sq[:], zh[:],
                                 start=False, stop=True)
                cdf_col = sm.tile([128, 2], F32)
                nc.vector.tensor_copy(cdf_col[:], cdfc_psum[:])

                cdfr_psum = psp.tile([1, NB], F32)
                nc.tensor.matmul(cdfr_psum[:], hist_col[:, 0:1], tri0[:],
                                 start=True, stop=False)
                nc.tensor.matmul(cdfr_psum[:], hist_col[:, 1:2], tri1[:],
                                 start=False, stop=True)
                cdf_row = sm.tile([1, NB], F32)
                nc.vector.tensor_copy(cdf_row[:], cdfr_psum[:])

                # ---------- G at knots ----------
                num_psum = psp.tile([1, NK], F32)
                nc.tensor.matmul(num_psum[:], cdf_col[:, 0:1], wt0[:],
                                 start=True, stop=False)
                nc.tensor.matmul(num_psum[:], cdf_col[:, 1:2], wt1[:],
                                 start=False, stop=True)
                den_psum = psp.tile([1, NK], F32)
                nc.tensor.matmul(den_psum[:], onesf_col[:], wt0[:],
                                 start=True, stop=False)
                nc.tensor.matmul(den_psum[:], onesf_col[:], wt1[:],
                                 start=False, stop=True)

                rden = sm.tile([1, NK], F32)
                nc.vector.reciprocal(rden[:], den_psum[:])
                g_raw = sm.tile([1, NK], F32)
                nc.vector.tensor_tensor(g_raw[:], num_psum[:], rden[:],
                                        mybir.AluOpType.mult)
                c0 = cdf_row[:, 0:1]
                dnorm = sm.tile([1, 1], F32)
                nc.vector.tensor_tensor(dnorm[:], cdf_row[:, NB - 1:NB], c0,
                                        mybir.AluOpType.subtract)
                rnorm = sm.tile([1, 1], F32)
                nc.vector.reciprocal(rnorm[:], dnorm[:])
                g_row = sm.tile([1, NK], F32)
                nc.vector.tensor_scalar(g_row[:], g_raw[:], c0, rnorm[:],
                                        mybir.AluOpType.subtract,
                                        mybir.AluOpType.mult)

                # ---------- PWL coefficients ----------
                NW = NSEG - 1                      # knots m = 1..NSEG-1
                beta = sm.tile([1, NSEG], F32)
                nc.vector.tensor_tensor(beta[:], g_row[:, 1:NK], g_row[:, 0:NSEG],
                                        mybir.AluOpType.subtract)
                nc.vector.tensor_scalar(beta[:], beta[:], float(NSEG), None,
                                        mybir.AluOpType.mult)
                wsh = sm.tile([1, NW], F32)        # w~_m = w_m + S
                nc.vector.tensor_tensor(wsh[:], beta[:, 1:NSEG], beta[:, 0:NW],
                                        mybir.AluOpType.subtract)
                nc.vector.tensor_scalar(wsh[:], wsh[:], SHIFT, None,
                                        mybir.AluOpType.add)

                # coef row: [0:NW] w~ ; [NW:2NW] +w~ theta ; [2NW:3NW] -w~ theta
                # [120] beta0 ; [121] A = G0 - sum_{dve knots} w~ theta
                coef_row = sm.tile([1, 160], F32)
                nc.vector.memset(coef_row[:], 0.0)
                nc.vector.tensor_copy(coef_row[:, 0:NW], wsh[:])
                s2 = coef_row[:, NW:2 * NW]
                nc.vector.tensor_tensor(s2, wsh[:], th_row[:, 1:NSEG],
                                        mybir.AluOpType.mult)
                nc.vector.tensor_scalar(coef_row[:, 2 * NW:3 * NW], s2, -1.0,
                                        None, mybir.AluOpType.mult)
                s2d = sm.tile([1, 1], F32)         # sum of w~ theta over DVE knots
                nc.vector.tensor_reduce(
                    s2d[:], coef_row[:, NW + NSEG - ND - 1:NW + NSEG - 1],
                    mybir.AxisListType.X, mybir.AluOpType.add)
                nc.vector.tensor_copy(coef_row[:, 150:151], beta[:, 0:1])
                nc.vector.tensor_tensor(coef_row[:, 151:152], g_row[:, 0:1],
                                        s2d[:], mybir.AluOpType.subtract)

                coef_psum = psp.tile([128, 160], F32)
                nc.tensor.matmul(coef_psum[:], ones_row[:], coef_row[:],
                                 start=True, stop=True)
                coef = sm.tile([128, 160], F32)
                nc.vector.tensor_copy(coef[:], coef_psum[:])

                if stage == 2:
                    nc.sync.dma_start(
                        out_dram.ap()[0:160].rearrange("(a b) -> a b", a=1),
                        coef_row[:])
                else:
                    # ------------- phase 2: PWL evaluation -------------
                    # linear term: beta0 x + A
                    lin = tp.tile([128, NCOL], F32)
                    nc.vector.tensor_scalar(lin[:], x_sb[:],
                                            coef[:, 150:151], coef[:, 151:152],
                                            mybir.AluOpType.mult,
                                            mybir.AluOpType.add)
                    nc.vector.tensor_tensor(acc[:], acc[:], lin[:],
                                            mybir.AluOpType.add)
                    # DVE knots m = NSEG-ND..NSEG-1: max(w~ x, w~ theta)
                    for m in range(NSEG - ND, NSEG):
                        t = tp.tile([128, NCOL], F32)
                        nc.vector.tensor_scalar(t[:], x_sb[:],
                                                coef[:, m - 1:m],
                                                coef[:, NW + m - 1:NW + m],
                                                mybir.AluOpType.mult,
                                                mybir.AluOpType.max)
                        nc.vector.tensor_tensor(acc[:], acc[:], t[:],
                                                mybir.AluOpType.add)
                    # scalar-engine knots m = 1..NSEG-ND-1: relu(w~ x - w~ th)
                    for m in range(1, NSEG - ND):
                        t = tp.tile([128, NCOL], F32)
                        nc.scalar.activation(t[:], x_sb[:], AF.Relu,
                                             bias=coef[:, 2 * NW + m - 1:2 * NW + m],
                                             scale=coef[:, m - 1:m])
                        nc.vector.tensor_tensor(acc[:], acc[:], t[:],
                                                mybir.AluOpType.add)

                    nc.sync.dma_start(
                        out_dram.ap().rearrange("(p t) -> p t", p=128),
                        acc[:])
    nc.compile()
    return nc


_NC_CACHE = None


def _get_nc():
    global _NC_CACHE
    if _NC_CACHE is None:
        _NC_CACHE = build_nc()
    return _NC_CACHE


def _axon_device_reset():
    """Recover a wedged axon terminal (NRT_EXEC_UNIT_UNRECOVERABLE)."""
    try:
        import ctypes
        import jax
        jax.devices()
        lib = ctypes.CDLL("/opt/axon/libaxon_pjrt.so")
        if hasattr(lib, "axon_reset"):
            lib.axon_reset.restype = ctypes.c_int64
            lib.axon_reset()
    except Exception:
        pass


def kernel(x: np.ndarray) -> np.ndarray:
    assert x.shape == (B, 1, H, W), x.shape
    x = np.ascontiguousarray(np.asarray(x, dtype=np.float32))
    nc = _get_nc()
    in_maps = []
    for core in range(N_CORES):
        b, q = core // 4, core % 4
        shard = x[b, 0, q * 128:(q + 1) * 128, :].reshape(QUARTER)
        in_maps.append({"x": np.ascontiguousarray(shard)})
    try:
        res = run_bass_kernel_spmd(nc, in_maps, core_ids=list(range(N_CORES)))
    except Exception:
        _axon_device_reset()
        res = run_bass_kernel_spmd(nc, in_maps, core_ids=list(range(N_CORES)))
    out = np.empty((B, 1, H, W), np.float32)
    for core in range(N_CORES):
        b, q = core // 4, core % 4
        out[b, 0, q * 128:(q + 1) * 128, :] = \
            res.results[core]["out"].reshape(128, W)
    return out


# revision 14
# speedup vs baseline: 2.9688x; 1.0899x over previous
"""Trainium2 Bass kernel for nn_Equalize (soft histogram equalization).

Per core (8 cores, each owns a quarter of one of the 2 images):
  1. Histogram: most pixel columns one-hot binned to a 128-level fine
     grid (DVE is_equal, bf16) and contracted with a ones-lhsT matmul
     (PE); NACT columns instead evaluate the exact Gaussian KDE on the
     256-bin reference grid on the Scalar engine (derivative_erf).
  2. AllReduce both partial histograms across the 4 cores of each image.
  3. The fine one-hot histogram is smoothed onto the 256-bin grid with a
     Gaussian Toeplitz matmul and added to the KDE part -> reference
     khist; cdf via triangular matmuls.
  4. G at 49 knots m/48: G(t) = sum_j k(t-b_j) cdf_j / sum_j k(t-b_j),
     via Gaussian-weight matmuls; normalized to cdfn afterwards.
  5. Per-pixel output = PWL interp of G via a relu expansion evaluated
     on Scalar (relu(w~ x - w~ th)) and DVE (max(w~ x, w~ th)) with
     w~ = w + S > 0; the shift is removed exactly with the closed form
     S*(k x - (k^2+k)/96), k = floor(48 x).  Terms accumulate into a
     running DVE sum.  No gpsimd gather anywhere.
"""
import os
import numpy as np

import concourse.bass as bass
import concourse.mybir as mybir
import concourse.tile as tile
import concourse.bacc as bacc
from concourse.bass_utils import run_bass_kernel_spmd

F32 = mybir.dt.float32
I32 = mybir.dt.int32
I16 = mybir.dt.int16
BF16 = mybir.dt.bfloat16

B, H, W = 2, 512, 512
N_CORES = 8
QUARTER = H // 4 * W            # 65536 pixels per core
NCOL = QUARTER // 128           # 512 pixel columns
NB = 256                        # reference histogram bins j/255
NFB = 128                       # fine one-hot grid p/127
TAU = 0.01
SQC = float(np.sqrt(1.0 / (2.0 * TAU * TAU)))   # 70.71
NSEG = 40                       # PWL segments, knots at m/40
NK = NSEG + 1
ND = 4                          # knots evaluated on DVE (m = NSEG-ND..NSEG-1)
SHIFT = 2.0                     # relu weight positivity shift
NACT = int(os.environ.get("KERNEL_NACT", 144))   # columns on scalar engine
AF = mybir.ActivationFunctionType


def mk_ap(handle_ap, offset, pairs):
    import dataclasses
    return dataclasses.replace(handle_ap, offset=offset, ap=list(pairs))


def build_nc(stage=3):
    stage = int(os.environ.get("KERNEL_STAGE", stage))
    nc = bacc.Bacc()
    x_dram = nc.declare_dram_parameter("x", [QUARTER], F32, isOutput=False)
    out_dram = nc.declare_dram_parameter("out", [QUARTER], F32, isOutput=True)

    NRED = NFB + NB             # 384: fine one-hot row + kde row
    hrows_dram = nc.dram_tensor("hrows", [NRED], F32)
    hred_dram = nc.dram_tensor("hred", [NRED], F32)

    NOH = NCOL - NACT           # one-hot columns

    with tile.TileContext(nc) as tc:
        with (
            tc.tile_pool(name="big", bufs=1) as big,
            tc.tile_pool(name="oh", bufs=4) as ohp,
            tc.tile_pool(name="term", bufs=6) as tp,
            tc.tile_pool(name="small", bufs=1) as sm,
            tc.tile_pool(name="psum", bufs=1, space="PSUM") as psp,
        ):
            # ---------------- constants ----------------
            iota_i = sm.tile([128, NFB], I32)
            nc.gpsimd.iota(iota_i[:], pattern=[[1, NFB]], base=0, channel_multiplier=0)
            iotaB = sm.tile([128, NFB], BF16)
            nc.vector.tensor_copy(iotaB[:], iota_i[:])
            iota256_i = sm.tile([128, NB], I32)
            nc.gpsimd.iota(iota256_i[:], pattern=[[1, NB]], base=0,
                           channel_multiplier=0)
            iota256 = sm.tile([128, NB], F32)
            nc.vector.tensor_copy(iota256[:], iota256_i[:])

            ones_col = sm.tile([128, 1], BF16)
            nc.vector.memset(ones_col[:], 1.0)
            onesf_col = sm.tile([128, 1], F32)
            nc.vector.memset(onesf_col[:], 1.0)
            ones_row = sm.tile([1, 128], F32)
            nc.vector.memset(ones_row[:], 1.0)

            # triangular: tri0[p, j] = 1 if j >= p ; tri1: j >= p+128
            tri_i = sm.tile([128, NB], I16)
            nc.gpsimd.iota(tri_i[:], pattern=[[1, NB]], base=0, channel_multiplier=-1)
            tri0 = sm.tile([128, NB], F32)
            nc.vector.tensor_scalar(tri0[:], tri_i[:], 0.0, None, mybir.AluOpType.is_ge)
            tri1 = sm.tile([128, NB], F32)
            nc.vector.tensor_scalar(tri1[:], tri_i[:], 128.0, None, mybir.AluOpType.is_ge)

            identity = sm.tile([128, 128], F32)
            id_i = sm.tile([128, 128], I16)
            nc.gpsimd.iota(id_i[:], pattern=[[1, 128]], base=0, channel_multiplier=-1)
            nc.vector.tensor_scalar(identity[:], id_i[:], 0.0, None,
                                    mybir.AluOpType.is_equal)

            # Gaussian tiles via derivative_erf(z) = 2/sqrt(pi) exp(-z^2);
            # the 2/sqrt(pi) factor is common to Toeplitz, KDE and knot
            # weights, so it cancels in all normalizations.
            def gauss_tile(npart, nfree, base, ch_mult, step, scale):
                ti = sm.tile([npart, nfree], I32)
                nc.gpsimd.iota(ti[:], pattern=[[step, nfree]], base=base,
                               channel_multiplier=ch_mult)
                tf = sm.tile([npart, nfree], F32)
                nc.vector.tensor_copy(tf[:], ti[:])
                tg = sm.tile([npart, nfree], F32)
                nc.scalar.activation(tg[:], tf[:], AF.Derivative_Erf, scale=scale)
                return tg

            # Toeplitz fine->coarse: KT[p, j] = k(p/127 - j/255), x32385
            ktoep = gauss_tile(128, NB, 0, 255, -127, SQC / 32385.0)
            # knot weights: wt[p, m] = k(m/NSEG - b_p), x(255*NSEG)
            wt0 = gauss_tile(128, NK, 0, -NSEG, 255, SQC / (255.0 * NSEG))
            wt1 = gauss_tile(128, NK, -NSEG * 128, -NSEG, 255,
                             SQC / (255.0 * NSEG))

            # knot position row [1, NK]: theta_m = m / 48
            th_i = sm.tile([1, NK], I32)
            nc.gpsimd.iota(th_i[:], pattern=[[1, NK]], base=0, channel_multiplier=0)
            th_row = sm.tile([1, NK], F32)
            nc.vector.tensor_scalar(th_row[:], th_i[:], 1.0 / NSEG, None,
                                    mybir.AluOpType.mult)

            # ---------------- phase 0: prep ----------------
            x_sb = big.tile([128, NCOL], F32)
            nc.sync.dma_start(x_sb[:], x_dram.ap().rearrange("(p t) -> p t", p=128))

            qi = big.tile([128, NCOL], I32)
            nc.vector.tensor_scalar(qi[:], x_sb[:], float(NFB - 1), None,
                                    mybir.AluOpType.mult)
            qf = big.tile([128, NCOL], F32)
            nc.vector.tensor_copy(qf[:], qi[:])
            xs = big.tile([128, NCOL], F32)
            nc.vector.tensor_scalar(xs[:], x_sb[:], -SQC, None, mybir.AluOpType.mult)

            # phase-2 prep + Q-correction accumulator init (needs only x)
            u_sb = big.tile([128, NCOL], F32)
            nc.vector.tensor_scalar(u_sb[:], x_sb[:], float(NSEG), 0.5,
                                    mybir.AluOpType.mult, mybir.AluOpType.subtract)
            ki = big.tile([128, NCOL], I32)
            nc.vector.tensor_copy(ki[:], u_sb[:])
            kf = big.tile([128, NCOL], F32)
            nc.vector.tensor_copy(kf[:], ki[:])
            kx = big.tile([128, NCOL], F32)
            nc.vector.tensor_tensor(kx[:], kf[:], x_sb[:], mybir.AluOpType.mult)
            k2k = big.tile([128, NCOL], F32)
            nc.vector.tensor_tensor(k2k[:], kf[:], kf[:], mybir.AluOpType.mult)
            nc.vector.tensor_tensor(k2k[:], k2k[:], kf[:], mybir.AluOpType.add)
            acc = big.tile([128, NCOL], F32)
            nc.vector.tensor_scalar(acc[:], kx[:], -SHIFT, None, mybir.AluOpType.mult)
            qc2 = big.tile([128, NCOL], F32)
            nc.vector.tensor_scalar(qc2[:], k2k[:], SHIFT / (2.0 * NSEG), None,
                                    mybir.AluOpType.mult)
            nc.vector.tensor_tensor(acc[:], acc[:], qc2[:], mybir.AluOpType.add)

            # ---------------- phase 1: histograms ----------------
            # one-hot columns -> fine 128-bin psum row [1, 512] (4 cols/mm)
            oh_psum = psp.tile([1, 4 * NFB], F32)
            for c in range(0, NOH, 4):
                oh = ohp.tile([128, 4 * NFB], BF16)
                for s4 in range(4):
                    nc.vector.tensor_scalar(oh[:, s4 * NFB:(s4 + 1) * NFB],
                                            iotaB[:], qf[:, c + s4:c + s4 + 1],
                                            None, mybir.AluOpType.is_equal)
                nc.tensor.matmul(oh_psum[:], ones_col[:], oh[:],
                                 start=(c == 0), stop=(c == NOH - 4))

            # KDE columns -> coarse 256-bin psum row [1, 512] (2 cols/mm)
            if NACT > 0:
                kde_psum = psp.tile([1, 2 * NB], F32)
                for ci in range(0, NACT, 2):
                    kt = ohp.tile([128, 2 * NB], BF16)
                    c = NOH + ci
                    nc.scalar.activation(kt[:, 0:NB], iota256[:], AF.Derivative_Erf,
                                         bias=xs[:, c:c + 1], scale=SQC / 255.0)
                    nc.scalar.activation(kt[:, NB:2 * NB], iota256[:],
                                         AF.Derivative_Erf,
                                         bias=xs[:, c + 1:c + 2], scale=SQC / 255.0)
                    nc.tensor.matmul(kde_psum[:], ones_col[:], kt[:],
                                     start=(ci == 0), stop=(ci == NACT - 2))

            hrow = sm.tile([1, 2 * NB], F32)
            nc.vector.tensor_copy(hrow[:, 0:4 * NFB], oh_psum[:])
            rows = sm.tile([1, NRED], F32)
            nc.vector.tensor_tensor(hrow[:, 0:NFB], hrow[:, 0:NFB],
                                    hrow[:, NFB:2 * NFB], mybir.AluOpType.add)
            nc.vector.tensor_tensor(hrow[:, 2 * NFB:3 * NFB],
                                    hrow[:, 2 * NFB:3 * NFB],
                                    hrow[:, 3 * NFB:4 * NFB], mybir.AluOpType.add)
            nc.vector.tensor_tensor(rows[:, 0:NFB], hrow[:, 0:NFB],
                                    hrow[:, 2 * NFB:3 * NFB], mybir.AluOpType.add)
            if NACT > 0:
                nc.vector.tensor_copy(hrow[:], kde_psum[:])
                nc.vector.tensor_tensor(rows[:, NFB:NFB + NB], hrow[:, 0:NB],
                                        hrow[:, NB:2 * NB], mybir.AluOpType.add)
            else:
                nc.vector.memset(rows[:, NFB:NFB + NB], 0.0)
            nc.sync.dma_start(hrows_dram.ap(), rows[:])

            if stage == 1:
                nc.sync.dma_start(
                    out_dram.ap()[0:NRED].rearrange("(a b) -> a b", a=1), rows[:])
            else:
                # ---------- allreduce over the 4 cores of this image ----------
                nc.gpsimd.collective_compute(
                    "AllReduce",
                    mybir.AluOpType.add,
                    ins=[hrows_dram.ap().opt()],
                    outs=[hred_dram.ap().opt()],
                    replica_groups=[[0, 1, 2, 3], [4, 5, 6, 7]],
                )

                ohq_col = sm.tile([128, 1], F32)
                nc.sync.dma_start(ohq_col[:],
                                  mk_ap(hred_dram.ap(), 0, [[1, 128], [128, 1]]))
                kde_col = sm.tile([128, 2], F32)
                nc.sync.dma_start(kde_col[:],
                                  mk_ap(hred_dram.ap(), NFB, [[1, 128], [128, 2]]))

                # ---------- khist_col [128, 2] = KT @ onehot + kde ----------
                histc_psum = psp.tile([128, 2], F32)
                nc.tensor.matmul(histc_psum[:], identity[:], kde_col[:],
                                 start=True, stop=False)
                nc.tensor.matmul(histc_psum[:, 0:1], ktoep[:, 0:128], ohq_col[:],
                                 start=False, stop=False)
                nc.tensor.matmul(histc_psum[:, 1:2], ktoep[:, 128:256], ohq_col[:],
                                 start=False, stop=True)
                hist_col = sm.tile([128, 2], F32)
                nc.vector.tensor_copy(hist_col[:], histc_psum[:])

                # ---------- cdf column [128, 2] and row [1, 256] ----------
                zh = sm.tile([128, 2], F32)
                nc.vector.memset(zh[:], 0.0)
                nc.vector.tensor_copy(zh[:, 1:2], hist_col[:, 0:1])
                ones_sq = sm.tile([128, 128], F32)
                nc.vector.memset(ones_sq[:], 1.0)
                cdfc_psum = psp.tile([128, 2], F32)
                nc.tensor.matmul(cdfc_psum[:], tri0[:, 0:128], hist_col[:],
                                 start=True, stop=False)
                nc.tensor.matmul(cdfc_psum[:], ones_sq[:], zh[:],
                                 start=False, stop=True)
                cdf_col = sm.tile([128, 2], F32)
                nc.vector.tensor_copy(cdf_col[:], cdfc_psum[:])

                cdfr_psum = psp.tile([1, NB], F32)
                nc.tensor.matmul(cdfr_psum[:], hist_col[:, 0:1], tri0[:],
                                 start=True, stop=False)
                nc.tensor.matmul(cdfr_psum[:], hist_col[:, 1:2], tri1[:],
                                 start=False, stop=True)
                cdf_row = sm.tile([1, NB], F32)
                nc.vector.tensor_copy(cdf_row[:], cdfr_psum[:])

                # ---------- G at knots ----------
                num_psum = psp.tile([1, NK], F32)
                nc.tensor.matmul(num_psum[:], cdf_col[:, 0:1], wt0[:],
                                 start=True, stop=False)
                nc.tensor.matmul(num_psum[:], cdf_col[:, 1:2], wt1[:],
                                 start=False, stop=True)
                den_psum = psp.tile([1, NK], F32)
                nc.tensor.matmul(den_psum[:], onesf_col[:], wt0[:],
                                 start=True, stop=False)
                nc.tensor.matmul(den_psum[:], onesf_col[:], wt1[:],
                                 start=False, stop=True)

                rden = sm.tile([1, NK], F32)
                nc.vector.reciprocal(rden[:], den_psum[:])
                g_raw = sm.tile([1, NK], F32)
                nc.vector.tensor_tensor(g_raw[:], num_psum[:], rden[:],
                                        mybir.AluOpType.mult)
                c0 = cdf_row[:, 0:1]
                dnorm = sm.tile([1, 1], F32)
                nc.vector.tensor_tensor(dnorm[:], cdf_row[:, NB - 1:NB], c0,
                                        mybir.AluOpType.subtract)
                rnorm = sm.tile([1, 1], F32)
                nc.vector.reciprocal(rnorm[:], dnorm[:])
                g_row = sm.tile([1, NK], F32)
                nc.vector.tensor_scalar(g_row[:], g_raw[:], c0, rnorm[:],
                                        mybir.AluOpType.subtract,
                                        mybir.AluOpType.mult)

                # ---------- PWL coefficients ----------
                NW = NSEG - 1                      # knots m = 1..NSEG-1
                beta = sm.tile([1, NSEG], F32)
                nc.vector.tensor_tensor(beta[:], g_row[:, 1:NK], g_row[:, 0:NSEG],
                                        mybir.AluOpType.subtract)
                nc.vector.tensor_scalar(beta[:], beta[:], float(NSEG), None,
                                        mybir.AluOpType.mult)
                wsh = sm.tile([1, NW], F32)        # w~_m = w_m + S
                nc.vector.tensor_tensor(wsh[:], beta[:, 1:NSEG], beta[:, 0:NW],
                                        mybir.AluOpType.subtract)
                nc.vector.tensor_scalar(wsh[:], wsh[:], SHIFT, None,
                                        mybir.AluOpType.add)

                # coef row: [0:NW] w~ ; [NW:2NW] +w~ theta ; [2NW:3NW] -w~ theta
                # [120] beta0 ; [121] A = G0 - sum_{dve knots} w~ theta
                coef_row = sm.tile([1, 160], F32)
                nc.vector.memset(coef_row[:], 0.0)
                nc.vector.tensor_copy(coef_row[:, 0:NW], wsh[:])
                s2 = coef_row[:, NW:2 * NW]
                nc.vector.tensor_tensor(s2, wsh[:], th_row[:, 1:NSEG],
                                        mybir.AluOpType.mult)
                nc.vector.tensor_scalar(coef_row[:, 2 * NW:3 * NW], s2, -1.0,
                                        None, mybir.AluOpType.mult)
                s2d = sm.tile([1, 1], F32)         # sum of w~ theta over DVE knots
                nc.vector.tensor_reduce(
                    s2d[:], coef_row[:, NW + NSEG - ND - 1:NW + NSEG - 1],
                    mybir.AxisListType.X, mybir.AluOpType.add)
                nc.vector.tensor_copy(coef_row[:, 150:151], beta[:, 0:1])
                nc.vector.tensor_tensor(coef_row[:, 151:152], g_row[:, 0:1],
                                        s2d[:], mybir.AluOpType.subtract)

                coef_psum = psp.tile([128, 160], F32)
                nc.tensor.matmul(coef_psum[:], ones_row[:], coef_row[:],
                                 start=True, stop=True)
                coef = sm.tile([128, 160], F32)
                nc.vector.tensor_copy(coef[:], coef_psum[:])

                if stage == 2:
                    nc.sync.dma_start(
                        out_dram.ap()[0:160].rearrange("(a b) -> a b", a=1),
                        coef_row[:])
                else:
                    # ------------- phase 2: PWL evaluation -------------
                    # linear term: beta0 x + A
                    lin = tp.tile([128, NCOL], F32)
                    nc.vector.tensor_scalar(lin[:], x_sb[:],
                                            coef[:, 150:151], coef[:, 151:152],
                                            mybir.AluOpType.mult,
                                            mybir.AluOpType.add)
                    nc.vector.tensor_tensor(acc[:], acc[:], lin[:],
                                            mybir.AluOpType.add)
                    # DVE knots m = NSEG-ND..NSEG-1: max(w~ x, w~ theta)
                    for m in range(NSEG - ND, NSEG):
                        t = tp.tile([128, NCOL], F32)
                        nc.vector.tensor_scalar(t[:], x_sb[:],
                                                coef[:, m - 1:m],
                                                coef[:, NW + m - 1:NW + m],
                                                mybir.AluOpType.mult,
                                                mybir.AluOpType.max)
                        nc.vector.tensor_tensor(acc[:], acc[:], t[:],
                                                mybir.AluOpType.add)
                    # scalar-engine knots m = 1..NSEG-ND-1: relu(w~ x - w~ th)
                    for m in range(1, NSEG - ND):
                        t = tp.tile([128, NCOL], F32)
                        nc.scalar.activation(t[:], x_sb[:], AF.Relu,
                                             bias=coef[:, 2 * NW + m - 1:2 * NW + m],
                                             scale=coef[:, m - 1:m])
                        nc.vector.tensor_tensor(acc[:], acc[:], t[:],
                                                mybir.AluOpType.add)

                    nc.sync.dma_start(
                        out_dram.ap().rearrange("(p t) -> p t", p=128),
                        acc[:])
    nc.compile()
    return nc


_NC_CACHE = None


def _get_nc():
    global _NC_CACHE
    if _NC_CACHE is None:
        _NC_CACHE = build_nc()
    return _NC_CACHE


def _axon_device_reset():
    """Recover a wedged axon terminal (NRT_EXEC_UNIT_UNRECOVERABLE)."""
    try:
        import ctypes
        import jax
        jax.devices()
        lib = ctypes.CDLL("/opt/axon/libaxon_pjrt.so")
        if hasattr(lib, "axon_reset"):
            lib.axon_reset.restype = ctypes.c_int64
            lib.axon_reset()
    except Exception:
        pass


def kernel(x: np.ndarray) -> np.ndarray:
    assert x.shape == (B, 1, H, W), x.shape
    x = np.ascontiguousarray(np.asarray(x, dtype=np.float32))
    nc = _get_nc()
    in_maps = []
    for core in range(N_CORES):
        b, q = core // 4, core % 4
        shard = x[b, 0, q * 128:(q + 1) * 128, :].reshape(QUARTER)
        in_maps.append({"x": np.ascontiguousarray(shard)})
    try:
        res = run_bass_kernel_spmd(nc, in_maps, core_ids=list(range(N_CORES)))
    except Exception:
        _axon_device_reset()
        res = run_bass_kernel_spmd(nc, in_maps, core_ids=list(range(N_CORES)))
    out = np.empty((B, 1, H, W), np.float32)
    for core in range(N_CORES):
        b, q = core // 4, core % 4
        out[b, 0, q * 128:(q + 1) * 128, :] = \
            res.results[core]["out"].reshape(128, W)
    return out
